# revision 1
# baseline (speedup 1.0000x reference)
"""GraphSAGE-mean + row-l2norm + normalized-linear classifier on 8 Trainium2
NeuronCores (Bass/Tile).

Sharding: target nodes split contiguously across 8 cores (12500 each); the
full x stays in every core's HBM as the gather table (weights replicated).
Per-edge source rows are fetched with dma_gather (SWDGE, 512B rows); the
scatter-add over destinations is done as one-hot selection matmuls on the
TensorEngine accumulating the feature-major mean aggregate per 256-node
window in PSUM:

    onehotT[e, j] = (dst_local[e] == j) * 1/deg[dst[e]]       (one DVE op)
    aggT_psum += msgs_block.T @ onehotT                       (one matmul)

Dense chain per window: hT = W_l.T @ aggT + W_r.T @ xT + b_l (bias via a K=1
ones matmul), row norms via a squared-column-sum matmul, and
out = (hT.T @ Wc_n) * rsqrt(sumsq) with the per-node scale applied where it is
a per-partition scalar.

Host prep: edges bucketed by (core, window, src-chunk) — 25000-row src chunks
keep dma_gather's int16 indices in range — padded to 128-edge blocks with
(idx=0, recip=0) slots.  Block counts are maxed over cores so all 8 cores run
one SPMD program.
"""
import sys
sys.path.insert(0, "/opt/trn_rl_repo")

import numpy as np

import concourse.bass as bass
import concourse.mybir as mybir
import concourse.tile as tile
from concourse import bacc, library_config
from concourse.bass_utils import run_bass_kernel_spmd

P = 128
EPS2 = 1e-24


def configure(n_nodes=100000, hid=128, num_cls=20, n_cores=8, w_win=256,
              gw=14, chunk_rows=25000, use_bf16=True):
    global N_NODES, HID, NUM_CLS, N_CORES, PER_CORE, W_WIN, GW, CHUNK_ROWS
    global N_CHUNKS, NW, NG, NT_PAD, USE_BF16
    N_NODES, HID, NUM_CLS, N_CORES = n_nodes, hid, num_cls, n_cores
    PER_CORE = n_nodes // n_cores
    W_WIN, GW, CHUNK_ROWS, USE_BF16 = w_win, gw, chunk_rows, use_bf16
    N_CHUNKS = (n_nodes + chunk_rows - 1) // chunk_rows
    NW = (PER_CORE + w_win - 1) // w_win
    NG = (NW + gw - 1) // gw
    NT_PAD = ((PER_CORE + P - 1) // P) * P


configure()


def _dt():
    return mybir.dt.bfloat16 if USE_BF16 else mybir.dt.float32


def _npdt():
    import ml_dtypes
    return ml_dtypes.bfloat16 if USE_BF16 else np.float32


def preprocess(x, edge_index, W_l, b_l, W_r, W_cls):
    """Host-side sharding/layout. Returns (in_maps, plan)."""
    src = np.asarray(edge_index[0], dtype=np.int64)
    dst = np.asarray(edge_index[1], dtype=np.int64)

    deg = np.bincount(dst, minlength=N_NODES).astype(np.float64)
    recip_all = (1.0 / np.maximum(deg, 1.0)).astype(np.float32)

    core = dst // PER_CORE
    ldst = dst % PER_CORE
    win = ldst // W_WIN
    chunk = src // CHUNK_ROWS

    key = (core * NW + win) * N_CHUNKS + chunk
    order = np.argsort(key, kind="stable")
    skey = key[order]

    nbuckets = N_CORES * NW * N_CHUNKS
    counts = np.bincount(skey, minlength=nbuckets).reshape(N_CORES, NW, N_CHUNKS)
    starts = np.zeros(nbuckets + 1, dtype=np.int64)
    np.cumsum(counts.reshape(-1), out=starts[1:])

    B = np.ceil(counts.max(axis=0) / P).astype(np.int64)        # [NW, N_CHUNKS]
    for w in range(NW):
        if B[w].sum() == 0:
            B[w, 0] = 1

    # block-column layout: group g -> chunk k -> windows w in group
    col_of = np.zeros((NW, N_CHUNKS), dtype=np.int64)
    grp_col0 = np.zeros(NG + 1, dtype=np.int64)
    c = 0
    for g in range(NG):
        grp_col0[g] = c
        for k in range(N_CHUNKS):
            for w in range(g * GW, min((g + 1) * GW, NW)):
                col_of[w, k] = c
                c += int(B[w, k])
    grp_col0[NG] = c
    C_TOT = c

    dt_np = _npdt()
    x32 = np.asarray(x, dtype=np.float32)
    x_src = np.ascontiguousarray(x32.astype(dt_np))
    Wc_n = np.asarray(W_cls, dtype=np.float32)
    Wc_n = Wc_n / np.maximum(np.sqrt((Wc_n * Wc_n).sum(0, keepdims=True)), 1e-12)

    in_maps = []
    for ci in range(N_CORES):
        idx_flat = np.zeros(C_TOT * P, dtype=np.int16)
        dst_flat = np.zeros(C_TOT * P, dtype=np.float32)
        rcp_flat = np.zeros(C_TOT * P, dtype=np.float32)
        for w in range(NW):
            for k in range(N_CHUNKS):
                b0 = starts[(ci * NW + w) * N_CHUNKS + k]
                b1 = starts[(ci * NW + w) * N_CHUNKS + k + 1]
                n = int(b1 - b0)
                if n == 0:
                    continue
                e = order[b0:b1]
                o = int(col_of[w, k]) * P
                idx_flat[o:o + n] = (src[e] - k * CHUNK_ROWS).astype(np.int16)
                dst_flat[o:o + n] = (ldst[e] - w * W_WIN).astype(np.float32)
                rcp_flat[o:o + n] = recip_all[dst[e]]
        base16 = idx_flat.reshape(-1, 16).T                     # [16, 8*C_TOT]
        idx16 = np.tile(base16, (8, 1))                         # [128, 8*C_TOT]
        dstp = dst_flat.reshape(C_TOT, P).T.copy()              # [128, C_TOT]
        rcpp = rcp_flat.reshape(C_TOT, P).T.copy()

        xT = np.zeros((HID, NT_PAD), dtype=dt_np)
        xT[:, :PER_CORE] = x32[ci * PER_CORE:(ci + 1) * PER_CORE].T.astype(dt_np)

        in_maps.append({
            "x_src": x_src,
            "idx16": np.ascontiguousarray(idx16),
            "dstp": np.ascontiguousarray(dstp),
            "rcpp": np.ascontiguousarray(rcpp),
            "xT": np.ascontiguousarray(xT),
            "W_l": np.asarray(W_l, dtype=np.float32).astype(dt_np),
            "W_r": np.asarray(W_r, dtype=np.float32).astype(dt_np),
            "blr": np.asarray(b_l, dtype=np.float32).astype(dt_np).reshape(1, HID),
            "Wc": Wc_n.astype(dt_np),
        })

    plan = {"B": B, "col_of": col_of, "grp_col0": grp_col0, "C_TOT": C_TOT}
    return in_maps, plan


def build(plan):
    B, col_of, grp_col0, C_TOT = plan["B"], plan["col_of"], plan["grp_col0"], plan["C_TOT"]
    dt = _dt()
    f32 = mybir.dt.float32

    nc = bacc.Bacc("TRN2", target_bir_lowering=False, debug=False,
                   enable_asserts=False)

    x_src = nc.dram_tensor("x_src", [N_NODES, HID], dt, kind="ExternalInput")
    idx16 = nc.dram_tensor("idx16", [P, 8 * C_TOT], mybir.dt.int16, kind="ExternalInput")
    dstp = nc.dram_tensor("dstp", [P, C_TOT], f32, kind="ExternalInput")
    rcpp = nc.dram_tensor("rcpp", [P, C_TOT], f32, kind="ExternalInput")
    xTd = nc.dram_tensor("xT", [HID, NT_PAD], dt, kind="ExternalInput")
    W_l = nc.dram_tensor("W_l", [HID, HID], dt, kind="ExternalInput")
    W_r = nc.dram_tensor("W_r", [HID, HID], dt, kind="ExternalInput")
    blr = nc.dram_tensor("blr", [1, HID], dt, kind="ExternalInput")
    Wc = nc.dram_tensor("Wc", [HID, NUM_CLS], dt, kind="ExternalInput")
    outd = nc.dram_tensor("out", [PER_CORE, NUM_CLS], f32, kind="ExternalOutput")

    xch = [x_src.ap()[k * CHUNK_ROWS:min((k + 1) * CHUNK_ROWS, N_NODES), :]
           for k in range(N_CHUNKS)]

    with tile.TileContext(nc) as tc:
        nc.gpsimd.load_library(library_config.mlp)
        with (
            tc.tile_pool(name="const", bufs=1) as cp,
            tc.tile_pool(name="grp", bufs=2) as gp,
            tc.tile_pool(name="win", bufs=2) as wp,
            tc.tile_pool(name="oh", bufs=4) as ohp,
            tc.tile_pool(name="sm", bufs=3) as sp,
            tc.tile_pool(name="pagg", bufs=2, space="PSUM") as pagg,
            tc.tile_pool(name="ph", bufs=2, space="PSUM") as php,
            tc.tile_pool(name="psm", bufs=2, space="PSUM") as psm,
        ):
            iota_i = cp.tile([P, W_WIN], mybir.dt.int32)
            nc.gpsimd.iota(iota_i[:], pattern=[[1, W_WIN]], base=0,
                           channel_multiplier=0)
            iota_dt = cp.tile([P, W_WIN], dt)
            nc.vector.tensor_copy(iota_dt[:], iota_i[:])
            ones_row = cp.tile([1, W_WIN], dt)
            nc.vector.memset(ones_row[:], 1.0)
            ones_col = cp.tile([P, 1], f32)
            nc.vector.memset(ones_col[:], 1.0)
            wl_t = cp.tile([HID, HID], dt)
            nc.sync.dma_start(out=wl_t[:], in_=W_l.ap())
            wr_t = cp.tile([HID, HID], dt)
            nc.sync.dma_start(out=wr_t[:], in_=W_r.ap())
            blr_t = cp.tile([1, HID], dt)
            nc.sync.dma_start(out=blr_t[:], in_=blr.ap())
            wc_t = cp.tile([HID, NUM_CLS], dt)
            nc.sync.dma_start(out=wc_t[:], in_=Wc.ap())

            for g in range(NG):
                c0, c1 = int(grp_col0[g]), int(grp_col0[g + 1])
                cg = c1 - c0
                ws = list(range(g * GW, min((g + 1) * GW, NW)))

                idx_t = gp.tile([P, 8 * cg], mybir.dt.int16, tag="idx")
                nc.sync.dma_start(out=idx_t[:], in_=idx16.ap()[:, 8 * c0:8 * c1])
                dst_t = gp.tile([P, cg], f32, tag="dst")
                nc.sync.dma_start(out=dst_t[:], in_=dstp.ap()[:, c0:c1])
                rcp_t = gp.tile([P, cg], f32, tag="rcp")
                nc.sync.dma_start(out=rcp_t[:], in_=rcpp.ap()[:, c0:c1])
                msgs = gp.tile([P, cg, HID], dt, tag="msgs")

                for k in range(N_CHUNKS):
                    kb = sum(int(B[w, k]) for w in ws)
                    if kb == 0:
                        continue
                    r0 = int(col_of[ws[0], k]) - c0
                    # cap calls at 64 blocks (8192 idx) - larger crashes HW
                    for s0 in range(0, kb, 64):
                        sn = min(64, kb - s0)
                        a = r0 + s0
                        nc.gpsimd.dma_gather(
                            out_ap=msgs[:, a:a + sn, :],
                            in_ap=xch[k],
                            idxs_ap=idx_t[:, 8 * a:8 * (a + sn)],
                            num_idxs=sn * P,
                            num_idxs_reg=sn * P,
                            elem_size=HID,
                            single_packet=False,
                        )

                for w in ws:
                    nb = w * W_WIN
                    wn = min(W_WIN, PER_CORE - nb)
                    ks = [k for k in range(N_CHUNKS) if B[w, k] > 0]
                    agg_ps = pagg.tile([P, W_WIN], f32, tag="agg")
                    for k in ks:
                        bk = int(B[w, k])
                        r0 = int(col_of[w, k]) - c0
                        for b in range(bk):
                            oh = ohp.tile([P, W_WIN], dt, tag="oh")
                            nc.vector.tensor_scalar(
                                out=oh[:], in0=iota_dt[:],
                                scalar1=dst_t[:, r0 + b:r0 + b + 1],
                                scalar2=rcp_t[:, r0 + b:r0 + b + 1],
                                op0=mybir.AluOpType.is_equal,
                                op1=mybir.AluOpType.mult,
                            )
                            nc.tensor.matmul(
                                out=agg_ps[:],
                                lhsT=msgs[:, r0 + b, :],
                                rhs=oh[:],
                                start=(k == ks[0] and b == 0),
                                stop=(k == ks[-1] and b == bk - 1),
                            )

                    aggT = wp.tile([P, W_WIN], dt, tag="aggT")
                    nc.scalar.copy(out=aggT[:], in_=agg_ps[:])

                    xT_t = wp.tile([HID, W_WIN], dt, tag="xT")
                    nc.sync.dma_start(out=xT_t[:], in_=xTd.ap()[:, nb:nb + W_WIN])

                    h_ps = php.tile([P, W_WIN], f32, tag="h")
                    nc.tensor.matmul(out=h_ps[:], lhsT=blr_t[:1, :],
                                     rhs=ones_row[:1, :], start=True, stop=False)
                    nc.tensor.matmul(out=h_ps[:], lhsT=wl_t[:], rhs=aggT[:],
                                     start=False, stop=False)
                    nc.tensor.matmul(out=h_ps[:], lhsT=wr_t[:], rhs=xT_t[:],
                                     start=False, stop=True)

                    hT = wp.tile([P, W_WIN], dt, tag="hT")
                    nc.scalar.copy(out=hT[:], in_=h_ps[:])
                    sq = wp.tile([P, W_WIN], f32, tag="sq")
                    nc.scalar.square(out=sq[:], in_=h_ps[:])

                    for hb in range((wn + P - 1) // P):
                        hw = min(P, wn - hb * P)
                        s_ps = psm.tile([P, 1], f32, tag="ss")
                        nc.tensor.matmul(out=s_ps[:hw, :],
                                         lhsT=sq[:, hb * P:hb * P + hw],
                                         rhs=ones_col[:, :], start=True, stop=True)
                        s_sb = sp.tile([P, 1], f32, tag="s")
                        nc.vector.tensor_scalar(out=s_sb[:hw, :], in0=s_ps[:hw, :],
                                                scalar1=EPS2, scalar2=None,
                                                op0=mybir.AluOpType.max)
                        r_sb = sp.tile([P, 1], f32, tag="r")
                        nc.vector.reciprocal(r_sb[:hw, :], s_sb[:hw, :])
                        rinv = sp.tile([P, 1], f32, tag="ri")
                        nc.scalar.sqrt(rinv[:hw, :], r_sb[:hw, :])

                        o_ps = psm.tile([P, NUM_CLS], f32, tag="op")
                        nc.tensor.matmul(out=o_ps[:hw, :],
                                         lhsT=hT[:, hb * P:hb * P + hw],
                                         rhs=wc_t[:], start=True, stop=True)
                        o_sb = sp.tile([P, NUM_CLS], f32, tag="ob")
                        nc.vector.tensor_scalar(out=o_sb[:hw, :], in0=o_ps[:hw, :],
                                                scalar1=rinv[:hw, :], scalar2=None,
                                                op0=mybir.AluOpType.mult)
                        nc.sync.dma_start(
                            out=outd.ap()[nb + hb * P: nb + hb * P + hw, :],
                            in_=o_sb[:hw, :])
    nc.compile()
    return nc


def kernel(x, edge_index, W_l, b_l, W_r, W_cls):
    in_maps, plan = preprocess(x, edge_index, W_l, b_l, W_r, W_cls)
    nc = build(plan)
    res = run_bass_kernel_spmd(nc, in_maps, core_ids=list(range(N_CORES)))
    out = np.concatenate([res.results[c]["out"] for c in range(N_CORES)], axis=0)
    return out.astype(np.float32)



# revision 5
# speedup vs baseline: 28.7918x; 28.7918x over previous
"""GraphSAGE-mean + row-l2norm + normalized-linear classifier on 8 Trainium2
NeuronCores (Bass/Tile).

Sharding: target nodes split contiguously across 8 cores (12500 each); the
full x stays in every core's HBM as the gather table (weights replicated).
Per-edge source rows are fetched with dma_gather (SWDGE, 512B rows); the
scatter-add over destinations is done as one-hot selection matmuls on the
TensorEngine accumulating the feature-major mean aggregate per 256-node
window in PSUM:

    onehotT[e, j] = (dst_local[e] == j) * 1/deg[dst[e]]       (one DVE op)
    aggT_psum += msgs_block.T @ onehotT                       (one matmul)

Dense chain per window: hT = W_l.T @ aggT + W_r.T @ xT + b_l (bias via a K=1
ones matmul), row norms via a squared-column-sum matmul, and
out = (hT.T @ Wc_n) * rsqrt(sumsq) with the per-node scale applied where it is
a per-partition scalar.

Host prep: edges bucketed by (core, window, src-chunk) — 25000-row src chunks
keep dma_gather's int16 indices in range — padded to 128-edge blocks with
(idx=0, recip=0) slots.  Block counts are maxed over cores so all 8 cores run
one SPMD program.
"""
import sys
sys.path.insert(0, "/opt/trn_rl_repo")

import numpy as np

import concourse.bass as bass
import concourse.mybir as mybir
import concourse.tile as tile
from concourse import bacc, library_config
from concourse.bass_utils import run_bass_kernel_spmd

P = 128
EPS2 = 1e-24


def configure(n_nodes=100000, hid=128, num_cls=20, n_cores=8, w_win=256,
              gw=14, chunk_rows=25000, use_bf16=True):
    global N_NODES, HID, NUM_CLS, N_CORES, PER_CORE, W_WIN, GW, CHUNK_ROWS
    global N_CHUNKS, NW, NG, NT_PAD, USE_BF16
    N_NODES, HID, NUM_CLS, N_CORES = n_nodes, hid, num_cls, n_cores
    PER_CORE = n_nodes // n_cores
    W_WIN, GW, CHUNK_ROWS, USE_BF16 = w_win, gw, chunk_rows, use_bf16
    N_CHUNKS = (n_nodes + chunk_rows - 1) // chunk_rows
    NW = (PER_CORE + w_win - 1) // w_win
    NG = (NW + gw - 1) // gw
    NT_PAD = ((PER_CORE + P - 1) // P) * P


configure()


def _dt():
    return mybir.dt.bfloat16 if USE_BF16 else mybir.dt.float32


def _npdt():
    import ml_dtypes
    return ml_dtypes.bfloat16 if USE_BF16 else np.float32


def preprocess(x, edge_index, W_l, b_l, W_r, W_cls):
    """Host-side sharding/layout. Returns (in_maps, plan)."""
    src = np.asarray(edge_index[0], dtype=np.int64)
    dst = np.asarray(edge_index[1], dtype=np.int64)

    deg = np.bincount(dst, minlength=N_NODES).astype(np.float64)
    recip_all = (1.0 / np.maximum(deg, 1.0)).astype(np.float32)

    core = dst // PER_CORE
    ldst = dst % PER_CORE
    win = ldst // W_WIN
    chunk = src // CHUNK_ROWS

    key = (core * NW + win) * N_CHUNKS + chunk
    order = np.argsort(key, kind="stable")
    skey = key[order]

    nbuckets = N_CORES * NW * N_CHUNKS
    counts = np.bincount(skey, minlength=nbuckets).reshape(N_CORES, NW, N_CHUNKS)
    starts = np.zeros(nbuckets + 1, dtype=np.int64)
    np.cumsum(counts.reshape(-1), out=starts[1:])

    B = np.ceil(counts.max(axis=0) / P).astype(np.int64)        # [NW, N_CHUNKS]
    for w in range(NW):
        if B[w].sum() == 0:
            B[w, 0] = 1

    # block-column layout: group g -> chunk k -> windows w in group
    col_of = np.zeros((NW, N_CHUNKS), dtype=np.int64)
    grp_col0 = np.zeros(NG + 1, dtype=np.int64)
    c = 0
    for g in range(NG):
        grp_col0[g] = c
        for k in range(N_CHUNKS):
            for w in range(g * GW, min((g + 1) * GW, NW)):
                col_of[w, k] = c
                c += int(B[w, k])
    grp_col0[NG] = c
    C_TOT = c

    dt_np = _npdt()
    x32 = np.asarray(x, dtype=np.float32)
    x_src = np.ascontiguousarray(x32.astype(dt_np))
    Wc_n = np.asarray(W_cls, dtype=np.float32)
    Wc_n = Wc_n / np.maximum(np.sqrt((Wc_n * Wc_n).sum(0, keepdims=True)), 1e-12)

    in_maps = []
    for ci in range(N_CORES):
        idx_flat = np.zeros(C_TOT * P, dtype=np.int16)
        dst_flat = np.zeros(C_TOT * P, dtype=np.float32)
        rcp_flat = np.zeros(C_TOT * P, dtype=np.float32)
        for w in range(NW):
            for k in range(N_CHUNKS):
                b0 = starts[(ci * NW + w) * N_CHUNKS + k]
                b1 = starts[(ci * NW + w) * N_CHUNKS + k + 1]
                n = int(b1 - b0)
                if n == 0:
                    continue
                e = order[b0:b1]
                o = int(col_of[w, k]) * P
                idx_flat[o:o + n] = (src[e] - k * CHUNK_ROWS).astype(np.int16)
                dst_flat[o:o + n] = (ldst[e] - w * W_WIN).astype(np.float32)
                rcp_flat[o:o + n] = recip_all[dst[e]]
        base16 = idx_flat.reshape(-1, 16).T                     # [16, 8*C_TOT]
        idx16 = np.tile(base16, (8, 1))                         # [128, 8*C_TOT]
        dstp = dst_flat.reshape(C_TOT, P).T.copy()              # [128, C_TOT]
        rcpp = rcp_flat.reshape(C_TOT, P).T.copy()

        xT = np.zeros((HID, NT_PAD), dtype=dt_np)
        xT[:, :PER_CORE] = x32[ci * PER_CORE:(ci + 1) * PER_CORE].T.astype(dt_np)

        in_maps.append({
            "x_src": x_src,
            "idx16": np.ascontiguousarray(idx16),
            "dstp": np.ascontiguousarray(dstp),
            "rcpp": np.ascontiguousarray(rcpp),
            "xT": np.ascontiguousarray(xT),
            "W_l": np.asarray(W_l, dtype=np.float32).astype(dt_np),
            "W_r": np.asarray(W_r, dtype=np.float32).astype(dt_np),
            "blr": np.asarray(b_l, dtype=np.float32).astype(dt_np).reshape(1, HID),
            "Wc": Wc_n.astype(dt_np),
        })

    plan = {"B": B, "col_of": col_of, "grp_col0": grp_col0, "C_TOT": C_TOT}
    return in_maps, plan


def build(plan):
    B, col_of, grp_col0, C_TOT = plan["B"], plan["col_of"], plan["grp_col0"], plan["C_TOT"]
    dt = _dt()
    f32 = mybir.dt.float32

    nc = bacc.Bacc("TRN2", target_bir_lowering=False, debug=False,
                   enable_asserts=False)

    x_src = nc.dram_tensor("x_src", [N_NODES, HID], dt, kind="ExternalInput")
    idx16 = nc.dram_tensor("idx16", [P, 8 * C_TOT], mybir.dt.int16, kind="ExternalInput")
    dstp = nc.dram_tensor("dstp", [P, C_TOT], f32, kind="ExternalInput")
    rcpp = nc.dram_tensor("rcpp", [P, C_TOT], f32, kind="ExternalInput")
    xTd = nc.dram_tensor("xT", [HID, NT_PAD], dt, kind="ExternalInput")
    W_l = nc.dram_tensor("W_l", [HID, HID], dt, kind="ExternalInput")
    W_r = nc.dram_tensor("W_r", [HID, HID], dt, kind="ExternalInput")
    blr = nc.dram_tensor("blr", [1, HID], dt, kind="ExternalInput")
    Wc = nc.dram_tensor("Wc", [HID, NUM_CLS], dt, kind="ExternalInput")
    outd = nc.dram_tensor("out", [PER_CORE, NUM_CLS], f32, kind="ExternalOutput")

    xch = [x_src.ap()[k * CHUNK_ROWS:min((k + 1) * CHUNK_ROWS, N_NODES), :]
           for k in range(N_CHUNKS)]

    with tile.TileContext(nc) as tc:
        nc.gpsimd.load_library(library_config.mlp)
        with (
            tc.tile_pool(name="const", bufs=1) as cp,
            tc.tile_pool(name="grp", bufs=3) as gp,
            tc.tile_pool(name="win", bufs=2) as wp,
            tc.tile_pool(name="oh", bufs=4) as ohp,
            tc.tile_pool(name="sm", bufs=3) as sp,
            tc.tile_pool(name="pagg", bufs=2, space="PSUM") as pagg,
            tc.tile_pool(name="ph", bufs=2, space="PSUM") as php,
            tc.tile_pool(name="psm", bufs=2, space="PSUM") as psm,
        ):
            iota_i = cp.tile([P, W_WIN], mybir.dt.int32)
            nc.gpsimd.iota(iota_i[:], pattern=[[1, W_WIN]], base=0,
                           channel_multiplier=0)
            iota_dt = cp.tile([P, W_WIN], dt)
            nc.vector.tensor_copy(iota_dt[:], iota_i[:])
            ones_row = cp.tile([1, W_WIN], dt)
            nc.vector.memset(ones_row[:], 1.0)
            ones_col = cp.tile([P, 1], f32)
            nc.vector.memset(ones_col[:], 1.0)
            wl_t = cp.tile([HID, HID], dt)
            nc.sync.dma_start(out=wl_t[:], in_=W_l.ap())
            wr_t = cp.tile([HID, HID], dt)
            nc.sync.dma_start(out=wr_t[:], in_=W_r.ap())
            blr_t = cp.tile([1, HID], dt)
            nc.sync.dma_start(out=blr_t[:], in_=blr.ap())
            wc_t = cp.tile([HID, NUM_CLS], dt)
            nc.sync.dma_start(out=wc_t[:], in_=Wc.ap())

            for g in range(NG):
                c0, c1 = int(grp_col0[g]), int(grp_col0[g + 1])
                cg = c1 - c0
                ws = list(range(g * GW, min((g + 1) * GW, NW)))

                idx_t = gp.tile([P, 8 * cg], mybir.dt.int16, tag="idx")
                nc.sync.dma_start(out=idx_t[:], in_=idx16.ap()[:, 8 * c0:8 * c1])
                dst_t = gp.tile([P, cg], f32, tag="dst")
                nc.sync.dma_start(out=dst_t[:], in_=dstp.ap()[:, c0:c1])
                rcp_t = gp.tile([P, cg], f32, tag="rcp")
                nc.sync.dma_start(out=rcp_t[:], in_=rcpp.ap()[:, c0:c1])
                msgs = gp.tile([P, cg, HID], dt, tag="msgs")

                for k in range(N_CHUNKS):
                    kb = sum(int(B[w, k]) for w in ws)
                    if kb == 0:
                        continue
                    r0 = int(col_of[ws[0], k]) - c0
                    # cap calls at 64 blocks (8192 idx) - larger crashes HW
                    for s0 in range(0, kb, 64):
                        sn = min(64, kb - s0)
                        a = r0 + s0
                        nc.gpsimd.dma_gather(
                            out_ap=msgs[:, a:a + sn, :],
                            in_ap=xch[k],
                            idxs_ap=idx_t[:, 8 * a:8 * (a + sn)],
                            num_idxs=sn * P,
                            num_idxs_reg=sn * P,
                            elem_size=HID,
                            single_packet=False,
                        )

                for w in ws:
                    nb = w * W_WIN
                    wn = min(W_WIN, PER_CORE - nb)
                    ks = [k for k in range(N_CHUNKS) if B[w, k] > 0]
                    agg_ps = pagg.tile([P, W_WIN], f32, tag="agg")
                    for k in ks:
                        bk = int(B[w, k])
                        r0 = int(col_of[w, k]) - c0
                        for b in range(bk):
                            oh = ohp.tile([P, W_WIN], dt, tag="oh")
                            nc.vector.tensor_scalar(
                                out=oh[:], in0=iota_dt[:],
                                scalar1=dst_t[:, r0 + b:r0 + b + 1],
                                scalar2=rcp_t[:, r0 + b:r0 + b + 1],
                                op0=mybir.AluOpType.is_equal,
                                op1=mybir.AluOpType.mult,
                            )
                            nc.tensor.matmul(
                                out=agg_ps[:],
                                lhsT=msgs[:, r0 + b, :],
                                rhs=oh[:],
                                start=(k == ks[0] and b == 0),
                                stop=(k == ks[-1] and b == bk - 1),
                            )

                    aggT = wp.tile([P, W_WIN], dt, tag="aggT")
                    nc.scalar.copy(out=aggT[:], in_=agg_ps[:])

                    xT_t = wp.tile([HID, W_WIN], dt, tag="xT")
                    nc.sync.dma_start(out=xT_t[:], in_=xTd.ap()[:, nb:nb + W_WIN])

                    h_ps = php.tile([P, W_WIN], f32, tag="h")
                    nc.tensor.matmul(out=h_ps[:], lhsT=blr_t[:1, :],
                                     rhs=ones_row[:1, :], start=True, stop=False)
                    nc.tensor.matmul(out=h_ps[:], lhsT=wl_t[:], rhs=aggT[:],
                                     start=False, stop=False)
                    nc.tensor.matmul(out=h_ps[:], lhsT=wr_t[:], rhs=xT_t[:],
                                     start=False, stop=True)

                    hT = wp.tile([P, W_WIN], dt, tag="hT")
                    nc.scalar.copy(out=hT[:], in_=h_ps[:])
                    sq = wp.tile([P, W_WIN], f32, tag="sq")
                    nc.scalar.square(out=sq[:], in_=h_ps[:])

                    for hb in range((wn + P - 1) // P):
                        hw = min(P, wn - hb * P)
                        s_ps = psm.tile([P, 1], f32, tag="ss")
                        nc.tensor.matmul(out=s_ps[:hw, :],
                                         lhsT=sq[:, hb * P:hb * P + hw],
                                         rhs=ones_col[:, :], start=True, stop=True)
                        s_sb = sp.tile([P, 1], f32, tag="s")
                        nc.vector.tensor_scalar(out=s_sb[:hw, :], in0=s_ps[:hw, :],
                                                scalar1=EPS2, scalar2=None,
                                                op0=mybir.AluOpType.max)
                        r_sb = sp.tile([P, 1], f32, tag="r")
                        nc.vector.reciprocal(r_sb[:hw, :], s_sb[:hw, :])
                        rinv = sp.tile([P, 1], f32, tag="ri")
                        nc.scalar.sqrt(rinv[:hw, :], r_sb[:hw, :])

                        o_ps = psm.tile([P, NUM_CLS], f32, tag="op")
                        nc.tensor.matmul(out=o_ps[:hw, :],
                                         lhsT=hT[:, hb * P:hb * P + hw],
                                         rhs=wc_t[:], start=True, stop=True)
                        o_sb = sp.tile([P, NUM_CLS], f32, tag="ob")
                        nc.vector.tensor_scalar(out=o_sb[:hw, :], in0=o_ps[:hw, :],
                                                scalar1=rinv[:hw, :], scalar2=None,
                                                op0=mybir.AluOpType.mult)
                        nc.sync.dma_start(
                            out=outd.ap()[nb + hb * P: nb + hb * P + hw, :],
                            in_=o_sb[:hw, :])
    nc.compile()
    return nc


def kernel(x, edge_index, W_l, b_l, W_r, W_cls):
    in_maps, plan = preprocess(x, edge_index, W_l, b_l, W_r, W_cls)
    nc = build(plan)
    res = run_bass_kernel_spmd(nc, in_maps, core_ids=list(range(N_CORES)))
    out = np.concatenate([res.results[c]["out"] for c in range(N_CORES)], axis=0)
    return out.astype(np.float32)



# revision 6
# speedup vs baseline: 31.7117x; 1.1014x over previous
"""GraphSAGE-mean + row-l2norm + normalized-linear classifier on 8 Trainium2
NeuronCores (Bass/Tile).

Sharding: target nodes split contiguously across 8 cores (12500 each); the
full x stays in every core's HBM as the gather table (weights replicated).
Per-edge source rows are fetched with dma_gather (SWDGE, 512B rows); the
scatter-add over destinations is done as one-hot selection matmuls on the
TensorEngine accumulating the feature-major mean aggregate per 256-node
window in PSUM:

    onehotT[e, j] = (dst_local[e] == j) * 1/deg[dst[e]]       (one DVE op)
    aggT_psum += msgs_block.T @ onehotT                       (one matmul)

Dense chain per window: hT = W_l.T @ aggT + W_r.T @ xT + b_l (bias via a K=1
ones matmul), row norms via a squared-column-sum matmul, and
out = (hT.T @ Wc_n) * rsqrt(sumsq) with the per-node scale applied where it is
a per-partition scalar.

Host prep: edges bucketed by (core, window, src-chunk) — 25000-row src chunks
keep dma_gather's int16 indices in range — padded to 128-edge blocks with
(idx=0, recip=0) slots.  Block counts are maxed over cores so all 8 cores run
one SPMD program.
"""
import sys
sys.path.insert(0, "/opt/trn_rl_repo")

import numpy as np

import concourse.bass as bass
import concourse.mybir as mybir
import concourse.tile as tile
from concourse import bacc, library_config
from concourse.bass_utils import run_bass_kernel_spmd

P = 128
EPS2 = 1e-24


def configure(n_nodes=100000, hid=128, num_cls=20, n_cores=8, w_win=256,
              gw=14, chunk_rows=25000, use_bf16=True):
    global N_NODES, HID, NUM_CLS, N_CORES, PER_CORE, W_WIN, GW, CHUNK_ROWS
    global N_CHUNKS, NW, NG, NT_PAD, USE_BF16
    N_NODES, HID, NUM_CLS, N_CORES = n_nodes, hid, num_cls, n_cores
    PER_CORE = n_nodes // n_cores
    W_WIN, GW, CHUNK_ROWS, USE_BF16 = w_win, gw, chunk_rows, use_bf16
    N_CHUNKS = (n_nodes + chunk_rows - 1) // chunk_rows
    NW = (PER_CORE + w_win - 1) // w_win
    NG = (NW + gw - 1) // gw
    NT_PAD = ((PER_CORE + P - 1) // P) * P


configure()


def _dt():
    return mybir.dt.bfloat16 if USE_BF16 else mybir.dt.float32


def _npdt():
    import ml_dtypes
    return ml_dtypes.bfloat16 if USE_BF16 else np.float32


def preprocess(x, edge_index, W_l, b_l, W_r, W_cls):
    """Host-side sharding/layout. Returns (in_maps, plan)."""
    src = np.asarray(edge_index[0], dtype=np.int64)
    dst = np.asarray(edge_index[1], dtype=np.int64)

    deg = np.bincount(dst, minlength=N_NODES).astype(np.float64)
    recip_all = (1.0 / np.maximum(deg, 1.0)).astype(np.float32)

    core = dst // PER_CORE
    ldst = dst % PER_CORE
    win = ldst // W_WIN
    chunk = src // CHUNK_ROWS

    key = (core * NW + win) * N_CHUNKS + chunk
    order = np.argsort(key, kind="stable")
    skey = key[order]

    nbuckets = N_CORES * NW * N_CHUNKS
    counts = np.bincount(skey, minlength=nbuckets).reshape(N_CORES, NW, N_CHUNKS)
    starts = np.zeros(nbuckets + 1, dtype=np.int64)
    np.cumsum(counts.reshape(-1), out=starts[1:])

    GRAN = 64
    U = np.ceil(counts.max(axis=0) / GRAN).astype(np.int64)     # [NW, N_CHUNKS]
    for w in range(NW):
        if U[w].sum() == 0:
            U[w, 0] = 1

    # 64-slot-unit layout: group g -> chunk k -> windows w in group.
    # Each (g,k) gather call starts 128-aligned (even unit position).
    unit_of = np.zeros((NW, N_CHUNKS), dtype=np.int64)
    grp_col0 = np.zeros(NG + 1, dtype=np.int64)
    call_c0 = np.zeros((NG, N_CHUNKS), dtype=np.int64)          # call start col
    call_nc = np.zeros((NG, N_CHUNKS), dtype=np.int64)          # call cols
    u = 0
    for g in range(NG):
        grp_col0[g] = u // 2
        for k in range(N_CHUNKS):
            call_c0[g, k] = u // 2
            for w in range(g * GW, min((g + 1) * GW, NW)):
                unit_of[w, k] = u
                u += int(U[w, k])
            if u % 2:
                u += 1
            call_nc[g, k] = u // 2 - call_c0[g, k]
    grp_col0[NG] = u // 2
    C_TOT = u // 2
    # base window per 128-column (window of the first slot in the column)
    wbase = np.zeros(C_TOT, dtype=np.int64)
    wbase_set = np.zeros(C_TOT, dtype=bool)
    for g in range(NG):
        for k in range(N_CHUNKS):
            for w in range(g * GW, min((g + 1) * GW, NW)):
                u0, nu = int(unit_of[w, k]), int(U[w, k])
                for uu in range(u0, u0 + nu):
                    c = uu // 2
                    if not wbase_set[c]:
                        wbase[c] = w
                        wbase_set[c] = True

    dt_np = _npdt()
    x32 = np.asarray(x, dtype=np.float32)
    x_src = np.ascontiguousarray(x32.astype(dt_np))
    Wc_n = np.asarray(W_cls, dtype=np.float32)
    Wc_n = Wc_n / np.maximum(np.sqrt((Wc_n * Wc_n).sum(0, keepdims=True)), 1e-12)

    in_maps = []
    for ci in range(N_CORES):
        idx_flat = np.zeros(C_TOT * P, dtype=np.int16)
        dst_flat = np.zeros(C_TOT * P, dtype=np.float32)
        rcp_flat = np.zeros(C_TOT * P, dtype=np.float32)
        for w in range(NW):
            for k in range(N_CHUNKS):
                b0 = starts[(ci * NW + w) * N_CHUNKS + k]
                b1 = starts[(ci * NW + w) * N_CHUNKS + k + 1]
                n = int(b1 - b0)
                if n == 0:
                    continue
                e = order[b0:b1]
                o = int(unit_of[w, k]) * 64
                idx_flat[o:o + n] = (src[e] - k * CHUNK_ROWS).astype(np.int16)
                slot_cols = (o + np.arange(n)) // P
                dst_flat[o:o + n] = (ldst[e] - wbase[slot_cols] * W_WIN
                                     ).astype(np.float32)
                rcp_flat[o:o + n] = recip_all[dst[e]]
        base16 = idx_flat.reshape(-1, 16).T                     # [16, 8*C_TOT]
        idx16 = np.tile(base16, (8, 1))                         # [128, 8*C_TOT]
        dstp = dst_flat.reshape(C_TOT, P).T.copy()              # [128, C_TOT]
        dstp2 = dstp - np.float32(W_WIN)
        rcpp = rcp_flat.reshape(C_TOT, P).T.copy()

        xT = np.zeros((HID, NT_PAD), dtype=dt_np)
        xT[:, :PER_CORE] = x32[ci * PER_CORE:(ci + 1) * PER_CORE].T.astype(dt_np)

        in_maps.append({
            "x_src": x_src,
            "idx16": np.ascontiguousarray(idx16),
            "dstp": np.ascontiguousarray(dstp),
            "dstp2": np.ascontiguousarray(dstp2),
            "rcpp": np.ascontiguousarray(rcpp),
            "xT": np.ascontiguousarray(xT),
            "W_l": np.asarray(W_l, dtype=np.float32).astype(dt_np),
            "W_r": np.asarray(W_r, dtype=np.float32).astype(dt_np),
            "blr": np.asarray(b_l, dtype=np.float32).astype(dt_np).reshape(1, HID),
            "Wc": Wc_n.astype(dt_np),
        })

    # per-window matmul pieces: (col, shifted) where shifted means the
    # window is the column's upper half (compare against dstp2)
    pieces = {}
    for w in range(NW):
        lst = []
        for k in range(N_CHUNKS):
            u0, nu = int(unit_of[w, k]), int(U[w, k])
            if counts.max(axis=0)[w, k] == 0:
                continue
            for c in range(u0 // 2, (u0 + nu + 1) // 2):
                sh = wbase[c] != w
                if sh:
                    assert w - wbase[c] == 1, (w, c, wbase[c])
                if (c, sh) not in lst:
                    lst.append((c, sh))
        pieces[w] = lst
    plan = {"grp_col0": grp_col0, "call_c0": call_c0, "call_nc": call_nc,
            "pieces": pieces, "C_TOT": C_TOT}
    return in_maps, plan


def build(plan):
    grp_col0, C_TOT = plan["grp_col0"], plan["C_TOT"]
    call_c0, call_nc, pieces = plan["call_c0"], plan["call_nc"], plan["pieces"]
    dt = _dt()
    f32 = mybir.dt.float32

    nc = bacc.Bacc("TRN2", target_bir_lowering=False, debug=False,
                   enable_asserts=False)

    x_src = nc.dram_tensor("x_src", [N_NODES, HID], dt, kind="ExternalInput")
    idx16 = nc.dram_tensor("idx16", [P, 8 * C_TOT], mybir.dt.int16, kind="ExternalInput")
    dstp = nc.dram_tensor("dstp", [P, C_TOT], f32, kind="ExternalInput")
    dstp2 = nc.dram_tensor("dstp2", [P, C_TOT], f32, kind="ExternalInput")
    rcpp = nc.dram_tensor("rcpp", [P, C_TOT], f32, kind="ExternalInput")
    xTd = nc.dram_tensor("xT", [HID, NT_PAD], dt, kind="ExternalInput")
    W_l = nc.dram_tensor("W_l", [HID, HID], dt, kind="ExternalInput")
    W_r = nc.dram_tensor("W_r", [HID, HID], dt, kind="ExternalInput")
    blr = nc.dram_tensor("blr", [1, HID], dt, kind="ExternalInput")
    Wc = nc.dram_tensor("Wc", [HID, NUM_CLS], dt, kind="ExternalInput")
    outd = nc.dram_tensor("out", [PER_CORE, NUM_CLS], f32, kind="ExternalOutput")

    xch = [x_src.ap()[k * CHUNK_ROWS:min((k + 1) * CHUNK_ROWS, N_NODES), :]
           for k in range(N_CHUNKS)]

    with tile.TileContext(nc) as tc:
        nc.gpsimd.load_library(library_config.mlp)
        with (
            tc.tile_pool(name="const", bufs=1) as cp,
            tc.tile_pool(name="grp", bufs=3) as gp,
            tc.tile_pool(name="win", bufs=2) as wp,
            tc.tile_pool(name="oh", bufs=4) as ohp,
            tc.tile_pool(name="sm", bufs=3) as sp,
            tc.tile_pool(name="pagg", bufs=2, space="PSUM") as pagg,
            tc.tile_pool(name="ph", bufs=2, space="PSUM") as php,
            tc.tile_pool(name="psm", bufs=2, space="PSUM") as psm,
        ):
            iota_i = cp.tile([P, W_WIN], mybir.dt.int32)
            nc.gpsimd.iota(iota_i[:], pattern=[[1, W_WIN]], base=0,
                           channel_multiplier=0)
            iota_dt = cp.tile([P, W_WIN], dt)
            nc.vector.tensor_copy(iota_dt[:], iota_i[:])
            ones_row = cp.tile([1, W_WIN], dt)
            nc.vector.memset(ones_row[:], 1.0)
            ones_col = cp.tile([P, 1], f32)
            nc.vector.memset(ones_col[:], 1.0)
            wl_t = cp.tile([HID, HID], dt)
            nc.sync.dma_start(out=wl_t[:], in_=W_l.ap())
            wr_t = cp.tile([HID, HID], dt)
            nc.sync.dma_start(out=wr_t[:], in_=W_r.ap())
            blr_t = cp.tile([1, HID], dt)
            nc.sync.dma_start(out=blr_t[:], in_=blr.ap())
            wc_t = cp.tile([HID, NUM_CLS], dt)
            nc.sync.dma_start(out=wc_t[:], in_=Wc.ap())

            for g in range(NG):
                c0, c1 = int(grp_col0[g]), int(grp_col0[g + 1])
                cg = c1 - c0
                ws = list(range(g * GW, min((g + 1) * GW, NW)))

                idx_t = gp.tile([P, 8 * cg], mybir.dt.int16, tag="idx")
                nc.sync.dma_start(out=idx_t[:], in_=idx16.ap()[:, 8 * c0:8 * c1])
                dst_t = gp.tile([P, cg], f32, tag="dst")
                nc.sync.dma_start(out=dst_t[:], in_=dstp.ap()[:, c0:c1])
                dst2_t = gp.tile([P, cg], f32, tag="dst2")
                nc.sync.dma_start(out=dst2_t[:], in_=dstp2.ap()[:, c0:c1])
                rcp_t = gp.tile([P, cg], f32, tag="rcp")
                nc.sync.dma_start(out=rcp_t[:], in_=rcpp.ap()[:, c0:c1])
                msgs = gp.tile([P, cg, HID], dt, tag="msgs")

                for k in range(N_CHUNKS):
                    kb = int(call_nc[g, k])
                    if kb == 0:
                        continue
                    r0 = int(call_c0[g, k]) - c0
                    # cap calls at 64 blocks (8192 idx) - larger crashes HW
                    for s0 in range(0, kb, 64):
                        sn = min(64, kb - s0)
                        a = r0 + s0
                        nc.gpsimd.dma_gather(
                            out_ap=msgs[:, a:a + sn, :],
                            in_ap=xch[k],
                            idxs_ap=idx_t[:, 8 * a:8 * (a + sn)],
                            num_idxs=sn * P,
                            num_idxs_reg=sn * P,
                            elem_size=HID,
                            single_packet=False,
                        )

                for w in ws:
                    nb = w * W_WIN
                    wn = min(W_WIN, PER_CORE - nb)
                    pcs = pieces[w]
                    agg_ps = pagg.tile([P, W_WIN], f32, tag="agg")
                    np_ = len(pcs)
                    for i, (col, sh) in enumerate(pcs):
                        rc = col - c0
                        dsrc = dst2_t if sh else dst_t
                        oh = ohp.tile([P, W_WIN], dt, tag="oh")
                        nc.vector.tensor_scalar(
                            out=oh[:], in0=iota_dt[:],
                            scalar1=dsrc[:, rc:rc + 1],
                            scalar2=rcp_t[:, rc:rc + 1],
                            op0=mybir.AluOpType.is_equal,
                            op1=mybir.AluOpType.mult,
                        )
                        nc.tensor.matmul(
                            out=agg_ps[:],
                            lhsT=msgs[:, rc, :],
                            rhs=oh[:],
                            start=(i == 0),
                            stop=(i == np_ - 1),
                        )

                    aggT = wp.tile([P, W_WIN], dt, tag="aggT")
                    nc.scalar.copy(out=aggT[:], in_=agg_ps[:])

                    xT_t = wp.tile([HID, W_WIN], dt, tag="xT")
                    nc.sync.dma_start(out=xT_t[:], in_=xTd.ap()[:, nb:nb + W_WIN])

                    h_ps = php.tile([P, W_WIN], f32, tag="h")
                    nc.tensor.matmul(out=h_ps[:], lhsT=blr_t[:1, :],
                                     rhs=ones_row[:1, :], start=True, stop=False)
                    nc.tensor.matmul(out=h_ps[:], lhsT=wl_t[:], rhs=aggT[:],
                                     start=False, stop=False)
                    nc.tensor.matmul(out=h_ps[:], lhsT=wr_t[:], rhs=xT_t[:],
                                     start=False, stop=True)

                    hT = wp.tile([P, W_WIN], dt, tag="hT")
                    nc.scalar.copy(out=hT[:], in_=h_ps[:])
                    sq = wp.tile([P, W_WIN], f32, tag="sq")
                    nc.scalar.square(out=sq[:], in_=h_ps[:])

                    for hb in range((wn + P - 1) // P):
                        hw = min(P, wn - hb * P)
                        s_ps = psm.tile([P, 1], f32, tag="ss")
                        nc.tensor.matmul(out=s_ps[:hw, :],
                                         lhsT=sq[:, hb * P:hb * P + hw],
                                         rhs=ones_col[:, :], start=True, stop=True)
                        s_sb = sp.tile([P, 1], f32, tag="s")
                        nc.vector.tensor_scalar(out=s_sb[:hw, :], in0=s_ps[:hw, :],
                                                scalar1=EPS2, scalar2=None,
                                                op0=mybir.AluOpType.max)
                        r_sb = sp.tile([P, 1], f32, tag="r")
                        nc.vector.reciprocal(r_sb[:hw, :], s_sb[:hw, :])
                        rinv = sp.tile([P, 1], f32, tag="ri")
                        nc.scalar.sqrt(rinv[:hw, :], r_sb[:hw, :])

                        o_ps = psm.tile([P, NUM_CLS], f32, tag="op")
                        nc.tensor.matmul(out=o_ps[:hw, :],
                                         lhsT=hT[:, hb * P:hb * P + hw],
                                         rhs=wc_t[:], start=True, stop=True)
                        o_sb = sp.tile([P, NUM_CLS], f32, tag="ob")
                        nc.vector.tensor_scalar(out=o_sb[:hw, :], in0=o_ps[:hw, :],
                                                scalar1=rinv[:hw, :], scalar2=None,
                                                op0=mybir.AluOpType.mult)
                        nc.sync.dma_start(
                            out=outd.ap()[nb + hb * P: nb + hb * P + hw, :],
                            in_=o_sb[:hw, :])
    nc.compile()
    return nc


def kernel(x, edge_index, W_l, b_l, W_r, W_cls):
    in_maps, plan = preprocess(x, edge_index, W_l, b_l, W_r, W_cls)
    nc = build(plan)
    res = run_bass_kernel_spmd(nc, in_maps, core_ids=list(range(N_CORES)))
    out = np.concatenate([res.results[c]["out"] for c in range(N_CORES)], axis=0)
    return out.astype(np.float32)

